# revision 28
# baseline (speedup 1.0000x reference)
"""HMM posterior kernel for Trainium2 (8 NeuronCores, SPMD data-parallel over batch).

Math: in the reference,
    ln_fs + ln_bs = (cs + ln_pi + t*ln_diag)
                  + (ln_pi + ln_emis[T-1] + (total - cs) + (T-1-t)*ln_diag)
                  = 2*ln_pi + ln_emis[:,T-1,:] + total + (T-1)*ln_diag
The cumsum terms cancel exactly, so the pre-normalization log_gamma is
independent of t, and so is its logsumexp over k.  The output is a [B, K]
tensor broadcast over the T axis.  With S1 = sum_t x, S2 = sum_t x^2,
xl = x[T-1], e = exp(-2*ls) (k- and b-constants cancel in the logsumexp):

    g[b,k] = A[b]*eh[k] + Bc[b]*r1[k] + r2[k]              (rank-3)
    A  = S2 + xl^2 - 2048   eh = -0.5*e          (A mean-shifted so its
    Bc = S1 + xl            r1 = e*mu             bf16 rounding is tiny)
    r2 = e*(-0.5*(T+1)*mu^2 - 1024) + kc
    kc = -(T+1)*ls + 2*pi + (T-1)*di             (a PE matmul over params)

g is ONE bf16 PE matmul of CC[128, BS] (rows 0/32/64/96 = A/Bc/1/1)
against RR[128, K] (rows 0/32/64/96 = eh/r1/r2_hi/r2_lo); r2 is split
hi+lo across two bf16 rows so the rank-3 product carries f32-level
accuracy; measured error is ~3e-3 against the 2e-2 gate.

Each core handles B/8 = 4 batch rows.  The kernel is output-write bound
(memory regime): the [4, T, K] shard is written in bf16 and widened to
f32 on the host.  Each row's gn is replicated x2 in SBUF (2 KB DMA
descriptors ~= HBM line rate) and the writes issue as 8 half-row DMAs
interleaved across the two HWDGE rings (sync + scalar) so both rings
drain evenly.
"""

import numpy as np

B, T, K = 32, 2048, 512
NCORES = 8
BS = B // NCORES  # 4 batch rows per core
W = 16            # t = p*W + w layout for the obvs stats pass
PW = T // W       # partitions used by the stats pass (128)
RJ = T // 128     # 16 t-rows per partition per batch row
REP = 2           # replication factor -> 2 KB DMA descriptors
JH = RJ // REP // 2  # half of the per-row DMA's j extent
LOG_2PI = float(np.log(2.0 * np.pi))
C = 0.5 * LOG_2PI

_BUILT = {}


def _build_nc(split_waits=True):
    key = ("nc", split_waits)
    if key in _BUILT:
        return _BUILT[key]

    from concourse import bass, tile
    import concourse.mybir as mybir

    f32 = mybir.dt.float32
    bf16 = mybir.dt.bfloat16
    AF = mybir.ActivationFunctionType
    ALU = mybir.AluOpType
    X = mybir.AxisListType.X

    nc = bass.Bass()
    obvs = nc.declare_dram_parameter("obvs", [BS, T], f32, isOutput=False)
    par4 = nc.declare_dram_parameter("par4", [2, K], f32, isOutput=False)
    ppb = nc.declare_dram_parameter("ppb", [4, K], f32, isOutput=False)
    cfkb = nc.declare_dram_parameter("cfkb", [4, 1], f32, isOutput=False)
    selc = nc.declare_dram_parameter("selc", [BS, BS * 128], bf16, isOutput=False)
    out = nc.declare_dram_parameter("out", [BS, T, K], bf16, isOutput=True)

    with tile.TileContext(nc) as tc:
        with (
            tc.tile_pool(name="sbuf", bufs=1) as pool,
            tc.tile_pool(name="psum", bufs=1, space="PSUM") as psum,
        ):
            # ---- all loads on the sync ring (ordered by first use) so the
            # scalar (ACT) ring is free: its activation-table load runs
            # during the DMA flight. ----
            pc = pool.tile([1, 2 * K], f32)
            nc.sync.dma_start(
                out=pc[:], in_=par4[:].rearrange("q k -> (q k)").unsqueeze(0)
            )
            obsq = pool.tile([PW, 2, BS, W], f32)
            nc.sync.dma_start(
                out=obsq[:, 0], in_=obvs[:].rearrange("b (p w) -> p b w", w=W)
            )
            pp = pool.tile([4, K], f32)
            nc.sync.dma_start(out=pp[:], in_=ppb[:])
            cfk = pool.tile([4, 1], f32)
            nc.sync.dma_start(out=cfk[:], in_=cfkb[:])
            xlr = pool.tile([1, BS], f32)
            nc.sync.dma_start(
                out=xlr[:], in_=obvs[:, T - 1 : T].rearrange("b one -> one b")
            )
            mu_r = pc[0:1, 0 * K : 1 * K]
            ls_r = pc[0:1, 1 * K : 2 * K]

            # ---- zero-padded bf16 matmul operands (memsets off the path) ----
            CC = pool.tile([128, BS], bf16)
            nc.vector.memset(CC[:], 0.0)
            nc.vector.memset(CC[64:65, :], 1.0)
            nc.vector.memset(CC[96:97, :], 1.0)
            RR = pool.tile([128, K], bf16)
            nc.vector.memset(RR[:], 0.0)
            ones_col = pool.tile([128, 1], f32)
            nc.vector.memset(ones_col[:], 1.0)

            # ---- ACT chain ----
            # The selector load warms the scalar HWDGE ring well before the
            # write phase (a cold ring costs ~3 us of DGE startup), but must
            # not get hoisted above the activation-table load: give it a WAR
            # dependency on an ACT write into the tile it overwrites.
            dmy = pool.tile([1, 1], f32)
            nc.vector.memset(dmy[:], 0.0)
            sel4 = pool.tile([BS, BS * 128], bf16)
            nc.scalar.copy(sel4[0:1, 0:1], dmy[:])
            nc.scalar.dma_start(out=sel4[:], in_=selc[:])
            nc.scalar.activation(obsq[:, 1], obsq[:, 0], AF.Square)
            er = pool.tile([1, K], f32)
            nc.scalar.activation(er[:], ls_r, AF.Exp, scale=-2.0)
            mu2r = pool.tile([1, K], f32)
            nc.scalar.activation(mu2r[:], mu_r, AF.Square)
            xl2 = pool.tile([1, BS], f32)
            nc.scalar.activation(xl2[:], xlr[:], AF.Square)
            # eh = -0.5*e -> RR row 0 (bf16 cast on output)
            nc.scalar.activation(RR[0:1, :], er[:], AF.Copy, scale=-0.5)

            # ---- PE: stats matmul + kc matmul (both off the DVE path) ----
            ps_s = psum.tile([1, 2 * BS * W], f32)
            nc.tensor.matmul(
                ps_s[:],
                lhsT=ones_col[0:PW, :],
                rhs=obsq[:].rearrange("p a b w -> p (a b w)"),
                start=True,
                stop=True,
            )
            kc_ps = psum.tile([1, K], f32, tag="kcps", name="kcps")
            nc.tensor.matmul(
                kc_ps[:], lhsT=cfk[:], rhs=pp[:], start=True, stop=True
            )

            # ---- DVE chain (program order = execution order) ----
            srow = pool.tile([1, 2 * BS], f32)
            nc.vector.reduce_sum(
                srow[:].unsqueeze(2),
                ps_s[:].rearrange("o (ab w) -> o ab w", w=W),
                axis=X,
            )
            # A = S2 + xl^2 - 2048 -> CC row 0 ; Bc = S1 + xl -> CC row 32
            nc.vector.scalar_tensor_tensor(
                out=CC[0:1, :], in0=srow[0:1, BS : 2 * BS], scalar=-2048.0,
                in1=xl2[:], op0=ALU.add, op1=ALU.add,
            )
            nc.vector.tensor_add(CC[32:33, :], srow[0:1, 0:BS], xlr[:])
            # r1 = e*mu -> RR row 32
            nc.vector.tensor_mul(RR[32:33, :], er[:], mu_r)
            # r2 = e*(-0.5*(T+1)*mu^2 - 1024) + kc, split hi/lo
            inner = pool.tile([1, K], f32)
            nc.vector.tensor_scalar(
                out=inner[:], in0=mu2r[:],
                scalar1=-0.5 * (float(T) + 1.0), scalar2=-1024.0,
                op0=ALU.mult, op1=ALU.add,
            )
            hm1 = pool.tile([1, K], f32)
            nc.vector.tensor_mul(hm1[:], inner[:], er[:])
            r2f = pool.tile([1, K], f32)
            nc.vector.tensor_add(r2f[:], hm1[:], kc_ps[:])
            # hi (bf16 cast) lands on p0 first — the verifier requires
            # equal base partitions for two SBUF inputs.  hi and lo stay on
            # DVE back-to-back (no cross-engine sem hop); ACT moves hi to
            # row 64 in parallel with lo.
            r2hi = pool.tile([1, K], bf16)
            nc.vector.tensor_copy(r2hi[:], r2f[:])
            nc.scalar.copy(RR[64:65, :], r2hi[:])
            nc.vector.tensor_tensor(
                RR[96:97, :], r2f[:], r2hi[:], ALU.subtract
            )

            # ---- g = CC^T @ RR (one bf16 PE matmul), then logsumexp ----
            g_ps = psum.tile([BS, K], f32, tag="gps", name="gps")
            nc.tensor.matmul(
                g_ps[:], lhsT=CC[:], rhs=RR[:], start=True, stop=True
            )
            negm = pool.tile([BS, 1], f32)
            nc.vector.reduce_max(negm[:], g_ps[:], axis=X, negate=True)
            et = pool.tile([BS, K], f32)
            s = pool.tile([BS, 1], f32)
            nc.scalar.activation(
                et[:], g_ps[:], AF.Exp, bias=negm[:], accum_out=s[:]
            )
            nls = pool.tile([BS, 1], f32)
            nc.scalar.activation(nls[:], s[:], AF.Ln)
            gn = pool.tile([BS, K], bf16)
            nc.vector.tensor_scalar(
                out=gn[:],
                in0=g_ps[:],
                scalar1=negm[:],
                scalar2=nls[:],
                op0=ALU.add,
                op1=ALU.subtract,
            )

            # ---- broadcast write: out[b, t, :] = gn[b, :] for all t ----
            # PE bf16 matmul replicates row b across 128 partitions; one DVE
            # + one ACT cast build the doubled [2K] block per partition
            # (2 KB descriptors); 8 half-row 1 MB stride-0 DMAs interleave
            # across the two HWDGE rings.
            bt2 = pool.tile([128, BS, REP * K], bf16)
            for b in range(BS):
                psB = psum.tile([128, K], f32, tag=f"psb{b}", name=f"psb{b}")
                nc.tensor.matmul(
                    psB[:],
                    lhsT=sel4[:, b * 128 : (b + 1) * 128],
                    rhs=gn[:],
                    start=True,
                    stop=True,
                )
                nc.vector.tensor_copy(
                    bt2[:, b, :].rearrange("p (r k) -> p r k", r=REP),
                    psB[:].unsqueeze(1).broadcast_to([128, REP, K]),
                )
                ov = out[b].rearrange(
                    "(p j r) k -> p j (r k)", j=RJ // REP, r=REP
                )
                for h in range(2):
                    eng = nc.sync if (2 * b + h) % 2 == 0 else nc.scalar
                    eng.dma_start(
                        out=ov[:, h * JH : (h + 1) * JH],
                        in_=bt2[:, b, :]
                        .unsqueeze(1)
                        .broadcast_to([128, JH, REP * K]),
                    )

    if split_waits:
        _split_multi_waits(nc, mybir)
    _BUILT[key] = nc
    return nc


def _split_multi_waits(nc, mybir):
    """This walrus build allows at most ONE sync wait per instruction.  Split
    any instruction with N>1 waits into N-1 single-wait NoOps on the same
    engine (executed immediately before it by the same sequencer) plus the
    original instruction carrying the final wait."""
    for fn in nc.m.functions:
        for blk in fn.blocks:
            new_insts = []
            for inst in blk.instructions:
                si = inst.sync_info
                if si is not None and len(si.on_wait) > 1:
                    waits = list(si.on_wait)
                    for i, w in enumerate(waits[:-1]):
                        new_insts.append(
                            mybir.InstNoOp(
                                name=f"{inst.name}-sw{i}",
                                engine=inst.engine,
                                sync_info=mybir.SyncInfo(
                                    on_wait=[w], on_update=[]
                                ),
                                bass_nofuse=True,
                            )
                        )
                    inst.sync_info = mybir.SyncInfo(
                        on_wait=[waits[-1]], on_update=list(si.on_update)
                    )
                new_insts.append(inst)
            blk.instructions = new_insts


def _host_constants():
    import ml_dtypes

    # kc coefficients for [mu, ls, pi, di]
    coefk = np.array(
        [[0.0], [-(float(T) + 1.0)], [2.0], [float(T - 1)]], dtype=np.float32
    )
    # row-replication selectors: selc[:, b*128:(b+1)*128] = e_b x ones(128)
    selc = np.zeros((BS, BS * 128), dtype=ml_dtypes.bfloat16)
    for b in range(BS):
        selc[b, b * 128 : (b + 1) * 128] = 1.0
    return np.ascontiguousarray(coefk), np.ascontiguousarray(selc)


def _run(inputs, trace=False, trace_kwargs=None):
    from concourse.bass_utils import run_bass_kernel_spmd

    nc = _build_nc()
    obvs = np.ascontiguousarray(np.asarray(inputs["obvs"], dtype=np.float32))
    import ml_dtypes

    par4 = np.ascontiguousarray(
        np.stack(
            [
                np.asarray(inputs["mu"], dtype=np.float32),
                np.asarray(inputs["log_sigma"], dtype=np.float32),
            ]
        )
    )
    ppb = np.ascontiguousarray(
        np.stack(
            [
                np.asarray(inputs["mu"], dtype=np.float32),
                np.asarray(inputs["log_sigma"], dtype=np.float32),
                np.asarray(inputs["ln_pi"], dtype=np.float32),
                np.asarray(inputs["ln_diag"], dtype=np.float32),
            ]
        )
    )
    coefk, selc = _host_constants()
    cfkb = np.ascontiguousarray(coefk)
    in_maps = [
        {
            "obvs": obvs[c * BS : (c + 1) * BS],
            "par4": par4,
            "ppb": ppb,
            "cfkb": cfkb,
            "selc": selc,
        }
        for c in range(NCORES)
    ]
    kw = {}
    if trace:
        kw["trace"] = True
        if trace_kwargs:
            kw["trace_kwargs"] = trace_kwargs
    res = run_bass_kernel_spmd(nc, in_maps, list(range(NCORES)), **kw)
    full = np.empty((B, T, K), dtype=np.float32)
    for c in range(NCORES):
        full[c * BS : (c + 1) * BS] = np.asarray(res.results[c]["out"]).astype(
            np.float32
        )
    return full, res


def kernel(**inputs) -> np.ndarray:
    full, _ = _run(inputs, trace=False)
    return full
